# revision 19
# baseline (speedup 1.0000x reference)
"""Masked dot-product attention on 8 Trainium2 NeuronCores — v2.

Problem: q,k,v [16, 2048, 128] fp32, valid_len [16] int -> out [16, 2048, 128].
out[b] = softmax(mask(q[b] @ k[b].T / sqrt(128), valid_len[b])) @ v[b]

v2 exploits that valid_len masks out entire 128-key tiles: only
ceil(valid_len/128) key tiles per batch contribute (127 of 256 total for the
graded input). Work is rebalanced across cores at KEY-TILE granularity using
flash-style partial attention:

  - Every core runs the SAME program (SPMD requirement): 3 "slot units" of
    fixed tile counts SLOTS=(8,6,4) = 18 virtual key tiles x 4 query passes.
  - A unit = (batch, contiguous range of live key tiles). The host assigns
    units to (core, slot) via bin packing and ships packed K/V/mask tiles plus
    that batch's Q^T per slot. Unused capacity is zero-padded (K=0 -> exp=1,
    V=0/mask=0 -> contributes nothing).
  - The device never normalizes: each unit outputs raw OT (V^T @ P^T partial)
    and its denominator row. The host merges: out_b = (sum OT_u) / (sum den_u)
    (exact — no max subtraction anywhere, so no rescaling is needed).
  - Denominator: each tile's mask matrix mb has its 0/1 mask in COLUMN s
    (slot id), so all 3 slots' denominator matmuls accumulate into a single
    shared PSUM bank (row s = den_s). PSUM = st(2 bufs x 2) + ot(3) + sbc(1)
    = exactly 8 banks.

Per-core per-pass pipeline (transposed layout, f32r matmuls, as v1):
    S^T_i = K_i^T.T @ Q_s^T      (PE, PSUM [k=128, q=512] x2 per pair)
    P^T_i = exp(S^T_i / sqrt(d)) (ScalarE, one [128,1024] inst per pair)
    OT_s += V_i.T  @ P^T_i       (PE accum, start/stop at slot bounds)
    den  += Mb_i.T @ P^T_i       (PE accum into shared bank, col-s banding)
    -> direct DMA store of OT_s (PSUM->DRAM) + den row per (slot, pass)
PV/sbc matmuls trail the score matmuls by 3 pairs (in-order PE queue keeps
busy while ACT computes exp); DMA tails deferred to pair 3 of the next pass
(Tile program-order: a read emitted before the trailing accumulating matmuls
would legally see a partial sum).
"""

import os

import numpy as np

import concourse.tile as tile
from concourse import bacc, mybir
from concourse.bass_utils import run_bass_kernel_spmd

B, SQ, SK, D = 16, 2048, 2048, 128
NCORES = 8
P = 128  # partitions
QW = 512  # query window (one PSUM bank)
NPASS = SQ // QW
SCALE = 1.0 / float(np.sqrt(D))

FP32 = mybir.dt.float32
F32R = mybir.dt.float32r
FP8 = mybir.dt.float8e4  # OCP e4m3, max 240
NPFP8 = mybir.dt.np(FP8)
# exp(s/sqrt(d) + EXP_BIAS): numerator and denominator scale identically, so
# the ratio is exact. -3 keeps e4m3 (max 240) finite up to 8.5-sigma scores
# (observed max 7.44) while keeping dominant weights in the normal range.
EXP_BIAS = -3.0

DEFAULT_SLOTS = (8, 6, 4)


# ---------------- unit packing (host) ----------------

def _pick_config(tiles):
    """Smallest-total 3-slot config that packs `tiles`. (16,16,16) always
    works (every batch fits one unit, 16 batches <= 24 units)."""
    cands = []
    for t1 in range(2, 17):
        for t2 in range(1, t1 + 1):
            for t3 in range(1, t2 + 1):
                if (t1 + t2 + t3) % 2 == 0:
                    cands.append((t1, t2, t3))
    cands.sort(key=lambda c: (sum(c), c[0]))
    for c in cands:
        a = _solve_units_slots(tiles, c)
        if a is not None:
            return c, a
    c = (16, 16, 16)
    return c, _solve_units_slots(tiles, c)


def _solve_units_slots(tiles, slots):
    """Like _solve_units but units are (slot_index) with possibly repeated
    sizes. Returns {batch: [(slot_idx, ntiles)]} using 8 units per slot."""
    nslots = len(slots)
    avail = [NCORES] * nslots
    assign = {b: [] for b in range(len(tiles))}
    order = sorted(range(nslots), key=lambda s: slots[s])  # ascending size
    multi = []
    for b in sorted(range(len(tiles)), key=lambda b: tiles[b]):
        t = tiles[b]
        if t == 0:
            continue
        fit = [s for s in order if slots[s] >= t and avail[s] > 0]
        if fit:
            s = fit[0]
            avail[s] -= 1
            assign[b].append((s, t))
        else:
            multi.append(b)
    for b in sorted(multi, key=lambda b: -tiles[b]):
        rem = tiles[b]
        while rem > 0:
            opts = [s for s in sorted(range(nslots), key=lambda s: -slots[s])
                    if avail[s] > 0]
            if not opts:
                return None
            cover = [s for s in order if slots[s] >= rem and avail[s] > 0]
            s = cover[0] if cover else opts[0]
            avail[s] -= 1
            take = min(slots[s], rem)
            assign[b].append((s, take))
            rem -= take
    return assign


# ---------------- device program ----------------

def _emit_loads(tc, ins, slots, big):
    """Queue input DMAs, ordered by first compute use. qt holds all slot Qs
    side by side [P, nslots*SQ]; kt/vs/mbs are packed virtual-tile layouts."""
    nc = tc.nc
    T = sum(slots)
    qT, kT, vm, mb = ins["qt"], ins["kt"], ins["vm"], ins["mb"]
    vm_r = vm.rearrange("(i p) d -> p i d", p=P)
    mb_r = mb.rearrange("(i p) d -> p i d", p=P)
    nslots = len(slots)
    qt = big.tile([P, nslots * SQ], F32R, tag="qt")
    kt = big.tile([P, T * P], F32R, tag="kt")
    vs = big.tile([P, T * P], F32R, tag="vs")
    mbs = big.tile([P, T * P], F32R, tag="mbs")

    def q_win(s, w):
        fs = slice(s * SQ + w * QW, s * SQ + (w + 1) * QW)
        nc.sync.dma_start(qt[:, fs], qT[:, fs])

    def k_chunk(c0, c1):
        fs = slice(c0 * P, c1 * P)
        nc.sync.dma_start(kt[:, fs], kT[:, fs])

    def vm_chunk(c0, c1):
        cs = slice(c0, c1)
        nc.sync.dma_start(vs.rearrange("p (i d) -> p i d", d=P)[:, cs], vm_r[:, cs])
        nc.sync.dma_start(mbs.rearrange("p (i d) -> p i d", d=P)[:, cs], mb_r[:, cs])

    # cold start: pair 0 needs kt tiles 0-1 + qt slot-0 window 0. Split them
    # into [P, 128] pieces so ~6 DMA queues move them in parallel (one queue
    # is only ~27 GB/s; a 256 KB chunk alone costs ~9.5 us).
    nc.sync.dma_start(kt[:, 0:P], kT[:, 0:P])
    for h in range(4):
        fs = slice(h * P, (h + 1) * P)
        nc.sync.dma_start(qt[:, fs], qT[:, fs])
    nc.sync.dma_start(kt[:, P : 2 * P], kT[:, P : 2 * P])
    nc.sync.dma_start(kt[:, 2 * P : 4 * P], kT[:, 2 * P : 4 * P])
    k_chunk(4, 8)
    q_win(1, 0)
    k_chunk(8, 12)
    q_win(2, 0)
    vm_chunk(0, 4)
    k_chunk(12, min(16, T))
    vm_chunk(4, 8)
    if T > 16:
        k_chunk(16, T)
    vm_chunk(8, 12)
    q_win(0, 1)
    vm_chunk(12, min(16, T))
    if T > 16:
        vm_chunk(16, T)
    q_win(1, 1)
    q_win(2, 1)
    for w in (2, 3):
        for s in range(nslots):
            q_win(s, w)
    return {"qt": qt, "kt": kt, "vs": vs, "mbs": mbs}


NWARM = 8  # fp32 warm matmuls are 4 cycles/row, ~0.9us each at full clock


def _emit_compute(tc, outs, slots, tiles, expb, warm_ops, ptp, tailp, psum, psacc):
    nc = tc.nc
    from collections import deque

    T = sum(slots)
    npair = T // 2
    nslots = len(slots)
    base = [sum(slots[:s]) for s in range(nslots)]
    last = [base[s] + slots[s] - 1 for s in range(nslots)]

    def slot_of(i):
        for s in range(nslots - 1, -1, -1):
            if i >= base[s]:
                return s
        return 0

    qt, kt, vs, mbs = tiles["qt"], tiles["kt"], tiles["vs"], tiles["mbs"]
    outot, outden = outs["outot"], outs["outden"]

    import heapq

    due = []  # (due_global_pair, seq, tail_fn)
    seq = 0
    pv_q = deque()
    pair_last = [last[s] // 2 for s in range(nslots)]

    def emit_pv(ots, sbc, pair, pt):
        for j in range(2):
            i = 2 * pair + j
            s = slot_of(i)
            psl = slice(j * QW, (j + 1) * QW)
            nc.tensor.matmul(
                ots[s],
                lhsT=vs[:, i * P : (i + 1) * P],
                rhs=pt[:, psl],
                start=(i == base[s]),
                stop=(i == last[s]),
            )
            nc.tensor.matmul(
                sbc,
                lhsT=mbs[:, i * P : (i + 1) * P],
                rhs=pt[:, psl],
                start=(i == 0),
                stop=(i == T - 1),
            )

    def tail_slot(s, qsl, ots, sbc):
        # PSUM cannot DMA directly to DRAM: bounce through SBUF on the
        # (otherwise idle) DVE, then store in 64KB pieces so the store
        # spreads over 4 DMA queues instead of ~9.5us on one.
        def fn():
            on = tailp.tile([P, QW], FP32, tag=f"on{s}", name=f"on{s}")
            nc.vector.tensor_scalar_add(on, ots[s], 0.0)
            for h in range(4):
                fs = slice(h * P, (h + 1) * P)
                osl = slice(qsl.start + h * P, qsl.start + (h + 1) * P)
                nc.sync.dma_start(outot[s][:, osl], on[:, fs])
            if s == nslots - 1:
                dsb = tailp.tile([nslots, QW], FP32, tag="dsb", name="dsb")
                nc.vector.tensor_scalar_add(dsb, sbc[0:nslots, :], 0.0)
                nc.sync.dma_start(outden[:, qsl], dsb)

        return fn

    # PE p-state warm-up: ~16 dependency-free matmuls on zero constants keep
    # the tensor engine continuously busy (targets the st pool; no readers)
    # so it reaches max clock before the first real score matmul.
    wl, wr = warm_ops
    for w in range(NWARM):
        stw = psum.tile([P, 2 * QW], FP32, tag="st", name=f"warm{w}")
        nc.tensor.matmul(stw[:, 0:QW], lhsT=wl, rhs=wr, start=True, stop=True)

    gp = 0  # global pair counter across passes
    for ip in range(NPASS):
        qsl = slice(ip * QW, (ip + 1) * QW)
        ots = [
            psacc.tile([P, QW], FP32, tag=f"ot{s}", name=f"ot{s}")
            for s in range(nslots)
        ]
        sbc = psacc.tile([P, QW], FP32, tag="sbc")
        for pair in range(npair):
            while due and due[0][0] <= gp:
                heapq.heappop(due)[2]()
            st = psum.tile([P, 2 * QW], FP32, tag="st")
            for j in range(2):
                i = 2 * pair + j
                s = slot_of(i)
                nc.tensor.matmul(
                    st[:, j * QW : (j + 1) * QW],
                    lhsT=kt[:, i * P : (i + 1) * P],
                    rhs=qt[:, s * SQ + ip * QW : s * SQ + (ip + 1) * QW],
                    start=True,
                    stop=True,
                )
            pt = ptp.tile([P, 2 * QW], F32R, tag="pt")
            nc.scalar.activation(
                pt, st, mybir.ActivationFunctionType.Exp,
                bias=expb, scale=SCALE,
            )
            pv_q.append((ots, sbc, pair, pt))
            if len(pv_q) > 3:
                emit_pv(*pv_q.popleft())
            gp += 1

        # per-slot tails, due one pair after the slot's last PV matmul has
        # drained from pv_q (emitting earlier would read a partial PSUM sum
        # under Tile's program-order semantics)
        for s in range(nslots):
            heapq.heappush(
                due, (ip * npair + pair_last[s] + 4, seq, tail_slot(s, qsl, ots, sbc))
            )
            seq += 1

    while pv_q:
        emit_pv(*pv_q.popleft())
    while due:
        heapq.heappop(due)[2]()


def _build_kernel(ctx, tc, outs, ins, slots):
    nc = tc.nc
    consts = ctx.enter_context(tc.tile_pool(name="consts", bufs=1))
    big = ctx.enter_context(tc.tile_pool(name="big", bufs=1))
    ptp = ctx.enter_context(tc.tile_pool(name="ptp", bufs=6))
    tailp = ctx.enter_context(tc.tile_pool(name="tailp", bufs=2))
    psum = ctx.enter_context(tc.tile_pool(name="psum", bufs=2, space="PSUM"))
    psacc = ctx.enter_context(tc.tile_pool(name="psacc", bufs=1, space="PSUM"))

    # warm the ACT exp spline table during the initial DMA wait
    warm = consts.tile([P, 1], FP32)
    nc.vector.memset(warm, 0.0)
    nc.scalar.activation(warm, warm, mybir.ActivationFunctionType.Exp)
    expb = consts.tile([P, 1], FP32, name="expb")
    nc.vector.memset(expb, EXP_BIAS)
    # PE warm-up operands: dummy matmuls ramp the tensor engine's DVFS
    # p-state to max while the first input DMAs are still in flight.
    wl = consts.tile([P, P], FP32, name="wl")
    wr = consts.tile([P, QW], FP32, name="wr")
    nc.vector.memset(wl, 0.0)
    nc.vector.memset(wr, 0.0)

    tiles = _emit_loads(tc, ins, slots, big)
    _emit_compute(tc, outs, slots, tiles, expb, (wl, wr), ptp, tailp, psum, psacc)


_NC_CACHE = {}


def _get_nc(slots):
    if slots in _NC_CACHE:
        return _NC_CACHE[slots]
    from contextlib import ExitStack

    T = sum(slots)
    nslots = len(slots)
    nc = bacc.Bacc(
        "TRN2",
        target_bir_lowering=False,
        debug=False,
        enable_asserts=False,
        num_devices=NCORES,
    )
    ins = {
        "qt": nc.dram_tensor("qt", [D, nslots * SQ], F32R, kind="ExternalInput").ap(),
        "kt": nc.dram_tensor("kt", [D, T * P], F32R, kind="ExternalInput").ap(),
        "vm": nc.dram_tensor("vm", [T * P, D], F32R, kind="ExternalInput").ap(),
        "mb": nc.dram_tensor("mb", [T * P, D], F32R, kind="ExternalInput").ap(),
    }
    outs = {
        "outot": nc.dram_tensor(
            "outot", [nslots, D, SQ], FP32, kind="ExternalOutput"
        ).ap(),
        "outden": nc.dram_tensor(
            "outden", [nslots, SQ], FP32, kind="ExternalOutput"
        ).ap(),
    }
    with tile.TileContext(nc) as tc:
        with ExitStack() as ctx:
            _build_kernel(ctx, tc, outs, ins, slots)
    nc.compile()
    _NC_CACHE[slots] = nc
    return nc


LAST_RESULTS = None  # BassKernelResults of the last run (for test harness)


def kernel(q, k, v, valid_len):
    q = np.ascontiguousarray(np.asarray(q, dtype=np.float32))
    k = np.ascontiguousarray(np.asarray(k, dtype=np.float32))
    v = np.ascontiguousarray(np.asarray(v, dtype=np.float32))
    vl = np.asarray(valid_len).astype(np.int64)

    tiles = [int(min((t + P - 1) // P, SK // P)) for t in vl]
    slots, assign = _pick_config(tiles)
    nslots = len(slots)
    T = sum(slots)

    m = (np.arange(SK)[None, :] < vl[:, None]).astype(np.float32)  # [B, SK]
    vm_full = v * m[:, :, None]  # [B, SK, D]
    qT = np.swapaxes(q, 1, 2)  # [B, D, SQ]
    kT = np.swapaxes(k, 1, 2)  # [B, D, SK]

    # ---- unit allocation: slot s of core c, cores assigned in order ----
    slot_cursor = [0] * nslots
    qtA = np.zeros((NCORES, D, nslots * SQ), np.float32)
    ktA = np.zeros((NCORES, D, T * P), np.float32)
    vmA = np.zeros((NCORES, T * P, D), np.float32)
    mbA = np.zeros((NCORES, T * P, D), np.float32)
    base = [sum(slots[:s]) for s in range(nslots)]
    units = []  # (batch, core, slot)
    for b, parts in assign.items():
        k0 = 0
        for s, cnt in parts:
            c = slot_cursor[s]
            slot_cursor[s] += 1
            assert c < NCORES, "unit packing overflow"
            units.append((b, c, s))
            qtA[c][:, s * SQ : (s + 1) * SQ] = qT[b]
            for l in range(cnt):
                g = k0 + l
                vt = base[s] + l
                ktA[c][:, vt * P : (vt + 1) * P] = kT[b][:, g * P : (g + 1) * P]
                vmA[c][vt * P : (vt + 1) * P, :] = vm_full[b][g * P : (g + 1) * P, :]
                mbA[c][vt * P : (vt + 1) * P, s] = m[b][g * P : (g + 1) * P]
            k0 += cnt

    nc = _get_nc(slots)
    in_maps = [
        {"qt": qtA[c], "kt": ktA[c], "vm": vmA[c], "mb": mbA[c]}
        for c in range(NCORES)
    ]
    tr = int(os.environ.get("KERNEL_TRACE", "0"))
    res = run_bass_kernel_spmd(
        nc,
        in_maps,
        core_ids=list(range(NCORES)),
        trace=tr > 0,
        trace_cores=(list(range(NCORES)) if tr == 2 else [0]) if tr else None,
    )
    global LAST_RESULTS
    LAST_RESULTS = res

    # ---- host merge: out_b = (sum OT_u) / (sum den_u) ----
    out = np.empty((B, SQ, D), np.float32)
    for b in range(B):
        us = [(c, s) for (bb, c, s) in units if bb == b]
        if not us:
            out[b] = v[b].mean(axis=0, keepdims=True)
            continue
        OT = np.zeros((D, SQ), np.float64)
        den = np.zeros((SQ,), np.float64)
        for c, s in us:
            OT += res.results[c]["outot"][s]
            den += res.results[c]["outden"][s]
        out[b] = (OT / den[None, :]).T
    return out.astype(np.float32)


# revision 20
# speedup vs baseline: 1.0122x; 1.0122x over previous
"""Masked dot-product attention on 8 Trainium2 NeuronCores — v2.

Problem: q,k,v [16, 2048, 128] fp32, valid_len [16] int -> out [16, 2048, 128].
out[b] = softmax(mask(q[b] @ k[b].T / sqrt(128), valid_len[b])) @ v[b]

v2 exploits that valid_len masks out entire 128-key tiles: only
ceil(valid_len/128) key tiles per batch contribute (127 of 256 total for the
graded input). Work is rebalanced across cores at KEY-TILE granularity using
flash-style partial attention:

  - Every core runs the SAME program (SPMD requirement): 3 "slot units" of
    fixed tile counts SLOTS=(8,6,4) = 18 virtual key tiles x 4 query passes.
  - A unit = (batch, contiguous range of live key tiles). The host assigns
    units to (core, slot) via bin packing and ships packed K/V/mask tiles plus
    that batch's Q^T per slot. Unused capacity is zero-padded (K=0 -> exp=1,
    V=0/mask=0 -> contributes nothing).
  - The device never normalizes: each unit outputs raw OT (V^T @ P^T partial)
    and its denominator row. The host merges: out_b = (sum OT_u) / (sum den_u)
    (exact — no max subtraction anywhere, so no rescaling is needed).
  - Denominator: each tile's mask matrix mb has its 0/1 mask in COLUMN s
    (slot id), so all 3 slots' denominator matmuls accumulate into a single
    shared PSUM bank (row s = den_s). PSUM = st(2 bufs x 2) + ot(3) + sbc(1)
    = exactly 8 banks.

Per-core per-pass pipeline (transposed layout, f32r matmuls, as v1):
    S^T_i = K_i^T.T @ Q_s^T      (PE, PSUM [k=128, q=512] x2 per pair)
    P^T_i = exp(S^T_i / sqrt(d)) (ScalarE, one [128,1024] inst per pair)
    OT_s += V_i.T  @ P^T_i       (PE accum, start/stop at slot bounds)
    den  += Mb_i.T @ P^T_i       (PE accum into shared bank, col-s banding)
    -> direct DMA store of OT_s (PSUM->DRAM) + den row per (slot, pass)
PV/sbc matmuls trail the score matmuls by 3 pairs (in-order PE queue keeps
busy while ACT computes exp); DMA tails deferred to pair 3 of the next pass
(Tile program-order: a read emitted before the trailing accumulating matmuls
would legally see a partial sum).
"""

import os

import numpy as np

import concourse.tile as tile
from concourse import bacc, mybir
from concourse.bass_utils import run_bass_kernel_spmd

B, SQ, SK, D = 16, 2048, 2048, 128
NCORES = 8
P = 128  # partitions
QW = 512  # query window (one PSUM bank)
NPASS = SQ // QW
SCALE = 1.0 / float(np.sqrt(D))

FP32 = mybir.dt.float32
F32R = mybir.dt.float32r
FP8 = mybir.dt.float8e4  # OCP e4m3, max 240
NPFP8 = mybir.dt.np(FP8)
# exp(s/sqrt(d) + EXP_BIAS): numerator and denominator scale identically, so
# the ratio is exact. -3 keeps e4m3 (max 240) finite up to 8.5-sigma scores
# (observed max 7.44) while keeping dominant weights in the normal range.
EXP_BIAS = -3.0

DEFAULT_SLOTS = (8, 6, 4)


# ---------------- unit packing (host) ----------------

def _pick_config(tiles):
    """Smallest-total 3-slot config that packs `tiles`. (16,16,16) always
    works (every batch fits one unit, 16 batches <= 24 units)."""
    cands = []
    for t1 in range(2, 17):
        for t2 in range(1, t1 + 1):
            for t3 in range(1, t2 + 1):
                if (t1 + t2 + t3) % 2 == 0:
                    cands.append((t1, t2, t3))
    cands.sort(key=lambda c: (sum(c), c[0]))
    for c in cands:
        a = _solve_units_slots(tiles, c)
        if a is not None:
            return c, a
    c = (16, 16, 16)
    return c, _solve_units_slots(tiles, c)


def _solve_units_slots(tiles, slots):
    """Like _solve_units but units are (slot_index) with possibly repeated
    sizes. Returns {batch: [(slot_idx, ntiles)]} using 8 units per slot."""
    nslots = len(slots)
    avail = [NCORES] * nslots
    assign = {b: [] for b in range(len(tiles))}
    order = sorted(range(nslots), key=lambda s: slots[s])  # ascending size
    multi = []
    for b in sorted(range(len(tiles)), key=lambda b: tiles[b]):
        t = tiles[b]
        if t == 0:
            continue
        fit = [s for s in order if slots[s] >= t and avail[s] > 0]
        if fit:
            s = fit[0]
            avail[s] -= 1
            assign[b].append((s, t))
        else:
            multi.append(b)
    for b in sorted(multi, key=lambda b: -tiles[b]):
        rem = tiles[b]
        while rem > 0:
            opts = [s for s in sorted(range(nslots), key=lambda s: -slots[s])
                    if avail[s] > 0]
            if not opts:
                return None
            cover = [s for s in order if slots[s] >= rem and avail[s] > 0]
            s = cover[0] if cover else opts[0]
            avail[s] -= 1
            take = min(slots[s], rem)
            assign[b].append((s, take))
            rem -= take
    return assign


# ---------------- device program ----------------

def _emit_loads(tc, ins, slots, big):
    """Queue input DMAs, ordered by first compute use. qt holds all slot Qs
    side by side [P, nslots*SQ]; kt/vs/mbs are packed virtual-tile layouts."""
    nc = tc.nc
    T = sum(slots)
    qT, kT, vm, mb = ins["qt"], ins["kt"], ins["vm"], ins["mb"]
    vm_r = vm.rearrange("(i p) d -> p i d", p=P)
    mb_r = mb.rearrange("(i p) d -> p i d", p=P)
    nslots = len(slots)
    qt = big.tile([P, nslots * SQ], F32R, tag="qt")
    kt = big.tile([P, T * P], F32R, tag="kt")
    vs = big.tile([P, T * P], F32R, tag="vs")
    mbs = big.tile([P, T * P], F32R, tag="mbs")

    def q_win(s, w):
        fs = slice(s * SQ + w * QW, s * SQ + (w + 1) * QW)
        nc.sync.dma_start(qt[:, fs], qT[:, fs])

    def k_chunk(c0, c1):
        fs = slice(c0 * P, c1 * P)
        nc.sync.dma_start(kt[:, fs], kT[:, fs])

    def vm_chunk(c0, c1):
        cs = slice(c0, c1)
        nc.sync.dma_start(vs.rearrange("p (i d) -> p i d", d=P)[:, cs], vm_r[:, cs])
        nc.sync.dma_start(mbs.rearrange("p (i d) -> p i d", d=P)[:, cs], mb_r[:, cs])

    # cold start: pair 0 needs kt tiles 0-1 + qt slot-0 window 0. Split them
    # into [P, 128] pieces so ~6 DMA queues move them in parallel (one queue
    # is only ~27 GB/s; a 256 KB chunk alone costs ~9.5 us).
    nc.sync.dma_start(kt[:, 0:P], kT[:, 0:P])
    for h in range(4):
        fs = slice(h * P, (h + 1) * P)
        nc.sync.dma_start(qt[:, fs], qT[:, fs])
    nc.sync.dma_start(kt[:, P : 2 * P], kT[:, P : 2 * P])
    nc.sync.dma_start(kt[:, 2 * P : 4 * P], kT[:, 2 * P : 4 * P])
    k_chunk(4, 8)
    q_win(1, 0)
    k_chunk(8, 12)
    q_win(2, 0)
    vm_chunk(0, 4)
    k_chunk(12, min(16, T))
    vm_chunk(4, 8)
    if T > 16:
        k_chunk(16, T)
    vm_chunk(8, 12)
    q_win(0, 1)
    vm_chunk(12, min(16, T))
    if T > 16:
        vm_chunk(16, T)
    q_win(1, 1)
    q_win(2, 1)
    for w in (2, 3):
        for s in range(nslots):
            q_win(s, w)
    return {"qt": qt, "kt": kt, "vs": vs, "mbs": mbs}


NWARM = 10  # small (64-wide fp32, 256c) so they never delay real work


def _emit_compute(tc, outs, slots, tiles, expb, warm_ops, ptp, tailp, psum, psacc):
    nc = tc.nc
    from collections import deque

    T = sum(slots)
    npair = T // 2
    nslots = len(slots)
    base = [sum(slots[:s]) for s in range(nslots)]
    last = [base[s] + slots[s] - 1 for s in range(nslots)]

    def slot_of(i):
        for s in range(nslots - 1, -1, -1):
            if i >= base[s]:
                return s
        return 0

    qt, kt, vs, mbs = tiles["qt"], tiles["kt"], tiles["vs"], tiles["mbs"]
    outot, outden = outs["outot"], outs["outden"]

    import heapq

    due = []  # (due_global_pair, seq, tail_fn)
    seq = 0
    pv_q = deque()
    pair_last = [last[s] // 2 for s in range(nslots)]

    def emit_pv(ots, sbc, pair, pt):
        for j in range(2):
            i = 2 * pair + j
            s = slot_of(i)
            psl = slice(j * QW, (j + 1) * QW)
            nc.tensor.matmul(
                ots[s],
                lhsT=vs[:, i * P : (i + 1) * P],
                rhs=pt[:, psl],
                start=(i == base[s]),
                stop=(i == last[s]),
            )
            nc.tensor.matmul(
                sbc,
                lhsT=mbs[:, i * P : (i + 1) * P],
                rhs=pt[:, psl],
                start=(i == 0),
                stop=(i == T - 1),
            )

    def tail_slot(s, qsl, ots, sbc):
        # PSUM cannot DMA directly to DRAM: bounce through SBUF on the
        # (otherwise idle) DVE, then store in 64KB pieces so the store
        # spreads over 4 DMA queues instead of ~9.5us on one.
        def fn():
            on = tailp.tile([P, QW], FP32, tag=f"on{s}", name=f"on{s}")
            nc.vector.tensor_scalar_add(on, ots[s], 0.0)
            for h in range(4):
                fs = slice(h * P, (h + 1) * P)
                osl = slice(qsl.start + h * P, qsl.start + (h + 1) * P)
                nc.sync.dma_start(outot[s][:, osl], on[:, fs])
            if s == nslots - 1:
                dsb = tailp.tile([nslots, QW], FP32, tag="dsb", name="dsb")
                nc.vector.tensor_scalar_add(dsb, sbc[0:nslots, :], 0.0)
                nc.sync.dma_start(outden[:, qsl], dsb)

        return fn

    # PE p-state warm-up: ~16 dependency-free matmuls on zero constants keep
    # the tensor engine continuously busy (targets the st pool; no readers)
    # so it reaches max clock before the first real score matmul.
    wl, wr = warm_ops
    for w in range(NWARM):
        stw = psum.tile([P, 2 * QW], FP32, tag="st", name=f"warm{w}")
        nc.tensor.matmul(stw[:, 0:64], lhsT=wl, rhs=wr, start=True, stop=True)

    gp = 0  # global pair counter across passes
    for ip in range(NPASS):
        qsl = slice(ip * QW, (ip + 1) * QW)
        ots = [
            psacc.tile([P, QW], FP32, tag=f"ot{s}", name=f"ot{s}")
            for s in range(nslots)
        ]
        sbc = psacc.tile([P, QW], FP32, tag="sbc")
        for pair in range(npair):
            while due and due[0][0] <= gp:
                heapq.heappop(due)[2]()
            st = psum.tile([P, 2 * QW], FP32, tag="st")
            for j in range(2):
                i = 2 * pair + j
                s = slot_of(i)
                nc.tensor.matmul(
                    st[:, j * QW : (j + 1) * QW],
                    lhsT=kt[:, i * P : (i + 1) * P],
                    rhs=qt[:, s * SQ + ip * QW : s * SQ + (ip + 1) * QW],
                    start=True,
                    stop=True,
                )
            pt = ptp.tile([P, 2 * QW], F32R, tag="pt")
            nc.scalar.activation(
                pt, st, mybir.ActivationFunctionType.Exp,
                bias=expb, scale=SCALE,
            )
            pv_q.append((ots, sbc, pair, pt))
            if len(pv_q) > 3:
                emit_pv(*pv_q.popleft())
            gp += 1

        # per-slot tails, due one pair after the slot's last PV matmul has
        # drained from pv_q (emitting earlier would read a partial PSUM sum
        # under Tile's program-order semantics)
        for s in range(nslots):
            heapq.heappush(
                due, (ip * npair + pair_last[s] + 4, seq, tail_slot(s, qsl, ots, sbc))
            )
            seq += 1

    while pv_q:
        emit_pv(*pv_q.popleft())
    while due:
        heapq.heappop(due)[2]()


def _build_kernel(ctx, tc, outs, ins, slots):
    nc = tc.nc
    consts = ctx.enter_context(tc.tile_pool(name="consts", bufs=1))
    big = ctx.enter_context(tc.tile_pool(name="big", bufs=1))
    ptp = ctx.enter_context(tc.tile_pool(name="ptp", bufs=6))
    tailp = ctx.enter_context(tc.tile_pool(name="tailp", bufs=2))
    psum = ctx.enter_context(tc.tile_pool(name="psum", bufs=2, space="PSUM"))
    psacc = ctx.enter_context(tc.tile_pool(name="psacc", bufs=1, space="PSUM"))

    # warm the ACT exp spline table during the initial DMA wait
    warm = consts.tile([P, 1], FP32)
    nc.vector.memset(warm, 0.0)
    nc.scalar.activation(warm, warm, mybir.ActivationFunctionType.Exp)
    expb = consts.tile([P, 1], FP32, name="expb")
    nc.vector.memset(expb, EXP_BIAS)
    # PE warm-up operands: dummy matmuls ramp the tensor engine's DVFS
    # p-state to max while the first input DMAs are still in flight.
    wl = consts.tile([P, P], FP32, name="wl")
    wr = consts.tile([P, 64], FP32, name="wr")
    nc.vector.memset(wl, 0.0)
    nc.vector.memset(wr, 0.0)

    tiles = _emit_loads(tc, ins, slots, big)
    _emit_compute(tc, outs, slots, tiles, expb, (wl, wr), ptp, tailp, psum, psacc)


_NC_CACHE = {}


def _get_nc(slots):
    if slots in _NC_CACHE:
        return _NC_CACHE[slots]
    from contextlib import ExitStack

    T = sum(slots)
    nslots = len(slots)
    nc = bacc.Bacc(
        "TRN2",
        target_bir_lowering=False,
        debug=False,
        enable_asserts=False,
        num_devices=NCORES,
    )
    ins = {
        "qt": nc.dram_tensor("qt", [D, nslots * SQ], F32R, kind="ExternalInput").ap(),
        "kt": nc.dram_tensor("kt", [D, T * P], F32R, kind="ExternalInput").ap(),
        "vm": nc.dram_tensor("vm", [T * P, D], F32R, kind="ExternalInput").ap(),
        "mb": nc.dram_tensor("mb", [T * P, D], F32R, kind="ExternalInput").ap(),
    }
    outs = {
        "outot": nc.dram_tensor(
            "outot", [nslots, D, SQ], FP32, kind="ExternalOutput"
        ).ap(),
        "outden": nc.dram_tensor(
            "outden", [nslots, SQ], FP32, kind="ExternalOutput"
        ).ap(),
    }
    with tile.TileContext(nc) as tc:
        with ExitStack() as ctx:
            _build_kernel(ctx, tc, outs, ins, slots)
    nc.compile()
    _NC_CACHE[slots] = nc
    return nc


LAST_RESULTS = None  # BassKernelResults of the last run (for test harness)


def kernel(q, k, v, valid_len):
    q = np.ascontiguousarray(np.asarray(q, dtype=np.float32))
    k = np.ascontiguousarray(np.asarray(k, dtype=np.float32))
    v = np.ascontiguousarray(np.asarray(v, dtype=np.float32))
    vl = np.asarray(valid_len).astype(np.int64)

    tiles = [int(min((t + P - 1) // P, SK // P)) for t in vl]
    slots, assign = _pick_config(tiles)
    nslots = len(slots)
    T = sum(slots)

    m = (np.arange(SK)[None, :] < vl[:, None]).astype(np.float32)  # [B, SK]
    vm_full = v * m[:, :, None]  # [B, SK, D]
    qT = np.swapaxes(q, 1, 2)  # [B, D, SQ]
    kT = np.swapaxes(k, 1, 2)  # [B, D, SK]

    # ---- unit allocation: slot s of core c, cores assigned in order ----
    slot_cursor = [0] * nslots
    qtA = np.zeros((NCORES, D, nslots * SQ), np.float32)
    ktA = np.zeros((NCORES, D, T * P), np.float32)
    vmA = np.zeros((NCORES, T * P, D), np.float32)
    mbA = np.zeros((NCORES, T * P, D), np.float32)
    base = [sum(slots[:s]) for s in range(nslots)]
    units = []  # (batch, core, slot)
    for b, parts in assign.items():
        k0 = 0
        for s, cnt in parts:
            c = slot_cursor[s]
            slot_cursor[s] += 1
            assert c < NCORES, "unit packing overflow"
            units.append((b, c, s))
            qtA[c][:, s * SQ : (s + 1) * SQ] = qT[b]
            for l in range(cnt):
                g = k0 + l
                vt = base[s] + l
                ktA[c][:, vt * P : (vt + 1) * P] = kT[b][:, g * P : (g + 1) * P]
                vmA[c][vt * P : (vt + 1) * P, :] = vm_full[b][g * P : (g + 1) * P, :]
                mbA[c][vt * P : (vt + 1) * P, s] = m[b][g * P : (g + 1) * P]
            k0 += cnt

    nc = _get_nc(slots)
    in_maps = [
        {"qt": qtA[c], "kt": ktA[c], "vm": vmA[c], "mb": mbA[c]}
        for c in range(NCORES)
    ]
    tr = int(os.environ.get("KERNEL_TRACE", "0"))
    res = run_bass_kernel_spmd(
        nc,
        in_maps,
        core_ids=list(range(NCORES)),
        trace=tr > 0,
        trace_cores=(list(range(NCORES)) if tr == 2 else [0]) if tr else None,
    )
    global LAST_RESULTS
    LAST_RESULTS = res

    # ---- host merge: out_b = (sum OT_u) / (sum den_u) ----
    out = np.empty((B, SQ, D), np.float32)
    for b in range(B):
        us = [(c, s) for (bb, c, s) in units if bb == b]
        if not us:
            out[b] = v[b].mean(axis=0, keepdims=True)
            continue
        OT = np.zeros((D, SQ), np.float64)
        den = np.zeros((SQ,), np.float64)
        for c, s in us:
            OT += res.results[c]["outot"][s]
            den += res.results[c]["outden"][s]
        out[b] = (OT / den[None, :]).T
    return out.astype(np.float32)


# revision 24
# speedup vs baseline: 1.0201x; 1.0078x over previous
"""Masked dot-product attention on 8 Trainium2 NeuronCores — v2.

Problem: q,k,v [16, 2048, 128] fp32, valid_len [16] int -> out [16, 2048, 128].
out[b] = softmax(mask(q[b] @ k[b].T / sqrt(128), valid_len[b])) @ v[b]

v2 exploits that valid_len masks out entire 128-key tiles: only
ceil(valid_len/128) key tiles per batch contribute (127 of 256 total for the
graded input). Work is rebalanced across cores at KEY-TILE granularity using
flash-style partial attention:

  - Every core runs the SAME program (SPMD requirement): 3 "slot units" of
    fixed tile counts SLOTS=(8,6,4) = 18 virtual key tiles x 4 query passes.
  - A unit = (batch, contiguous range of live key tiles). The host assigns
    units to (core, slot) via bin packing and ships packed K/V/mask tiles plus
    that batch's Q^T per slot. Unused capacity is zero-padded (K=0 -> exp=1,
    V=0/mask=0 -> contributes nothing).
  - The device never normalizes: each unit outputs raw OT (V^T @ P^T partial)
    and its denominator row. The host merges: out_b = (sum OT_u) / (sum den_u)
    (exact — no max subtraction anywhere, so no rescaling is needed).
  - Denominator: each tile's mask matrix mb has its 0/1 mask in COLUMN s
    (slot id), so all 3 slots' denominator matmuls accumulate into a single
    shared PSUM bank (row s = den_s). PSUM = st(2 bufs x 2) + ot(3) + sbc(1)
    = exactly 8 banks.

Per-core per-pass pipeline (transposed layout, f32r matmuls, as v1):
    S^T_i = K_i^T.T @ Q_s^T      (PE, PSUM [k=128, q=512] x2 per pair)
    P^T_i = exp(S^T_i / sqrt(d)) (ScalarE, one [128,1024] inst per pair)
    OT_s += V_i.T  @ P^T_i       (PE accum, start/stop at slot bounds)
    den  += Mb_i.T @ P^T_i       (PE accum into shared bank, col-s banding)
    -> direct DMA store of OT_s (PSUM->DRAM) + den row per (slot, pass)
PV/sbc matmuls trail the score matmuls by 3 pairs (in-order PE queue keeps
busy while ACT computes exp); DMA tails deferred to pair 3 of the next pass
(Tile program-order: a read emitted before the trailing accumulating matmuls
would legally see a partial sum).
"""

import os

import numpy as np

import concourse.tile as tile
from concourse import bacc, mybir
from concourse.bass_utils import run_bass_kernel_spmd

B, SQ, SK, D = 16, 2048, 2048, 128
NCORES = 8
P = 128  # partitions
QW = 512  # query window (one PSUM bank)
NPASS = SQ // QW
SCALE = 1.0 / float(np.sqrt(D))

FP32 = mybir.dt.float32
F32R = mybir.dt.float32r
FP8 = mybir.dt.float8e4  # OCP e4m3, max 240
NPFP8 = mybir.dt.np(FP8)
# exp(s/sqrt(d) + EXP_BIAS): numerator and denominator scale identically, so
# the ratio is exact. -3 keeps e4m3 (max 240) finite up to 8.5-sigma scores
# (observed max 7.44) while keeping dominant weights in the normal range.
EXP_BIAS = -3.0

DEFAULT_SLOTS = (8, 6, 4)


# ---------------- unit packing (host) ----------------

def _pick_config(tiles):
    """Smallest-total 3-slot config that packs `tiles`. (16,16,16) always
    works (every batch fits one unit, 16 batches <= 24 units)."""
    cands = []
    for t1 in range(2, 17):
        for t2 in range(1, t1 + 1):
            for t3 in range(1, t2 + 1):
                if (t1 + t2 + t3) % 2 == 0:
                    cands.append((t1, t2, t3))
    cands.sort(key=lambda c: (sum(c), c[0]))
    for c in cands:
        a = _solve_units_slots(tiles, c)
        if a is not None:
            return c, a
    c = (16, 16, 16)
    return c, _solve_units_slots(tiles, c)


def _solve_units_slots(tiles, slots):
    """Like _solve_units but units are (slot_index) with possibly repeated
    sizes. Returns {batch: [(slot_idx, ntiles)]} using 8 units per slot."""
    nslots = len(slots)
    avail = [NCORES] * nslots
    assign = {b: [] for b in range(len(tiles))}
    order = sorted(range(nslots), key=lambda s: slots[s])  # ascending size
    multi = []
    for b in sorted(range(len(tiles)), key=lambda b: tiles[b]):
        t = tiles[b]
        if t == 0:
            continue
        fit = [s for s in order if slots[s] >= t and avail[s] > 0]
        if fit:
            s = fit[0]
            avail[s] -= 1
            assign[b].append((s, t))
        else:
            multi.append(b)
    for b in sorted(multi, key=lambda b: -tiles[b]):
        rem = tiles[b]
        while rem > 0:
            opts = [s for s in sorted(range(nslots), key=lambda s: -slots[s])
                    if avail[s] > 0]
            if not opts:
                return None
            cover = [s for s in order if slots[s] >= rem and avail[s] > 0]
            s = cover[0] if cover else opts[0]
            avail[s] -= 1
            take = min(slots[s], rem)
            assign[b].append((s, take))
            rem -= take
    return assign


# ---------------- device program ----------------

def _emit_loads(tc, ins, slots, big):
    """Queue input DMAs, ordered by first compute use. All inputs are 2-D
    [P, X] contiguous, so each dma_start is one cheap HWDGE trigger (~0.6us
    on its sequencer) whose 128 descriptors spread over all 16 DMA engines.
    Triggers are split between the Sync queue (kt/qt: pass-0 critical) and
    the idle GpSimd queue (vs/mbs and late q windows)."""
    nc = tc.nc
    T = sum(slots)
    qT, kT, vm, mb = ins["qt"], ins["kt"], ins["vm"], ins["mb"]
    nslots = len(slots)
    qt = big.tile([P, nslots * SQ], F32R, tag="qt")
    kt = big.tile([P, T * P], F32R, tag="kt")
    vs = big.tile([P, T * P], F32R, tag="vs")
    mbs = big.tile([P, T * P], F32R, tag="mbs")

    def q_win(s, w, eng=None):
        fs = slice(s * SQ + w * QW, s * SQ + (w + 1) * QW)
        (eng or nc.sync).dma_start(qt[:, fs], qT[:, fs])

    def k_chunk(c0, c1):
        fs = slice(c0 * P, c1 * P)
        nc.sync.dma_start(kt[:, fs], kT[:, fs])

    def vm_chunk(c0, c1):
        fs = slice(c0 * P, c1 * P)
        nc.gpsimd.dma_start(vs[:, fs], vm[:, fs])
        nc.gpsimd.dma_start(mbs[:, fs], mb[:, fs])

    k_chunk(0, 4)
    q_win(0, 0)
    vm_chunk(0, T // 2)
    k_chunk(4, 12)
    q_win(1, 0)
    q_win(2, 0)
    vm_chunk(T // 2, T)
    k_chunk(12, T)
    q_win(0, 1)
    q_win(1, 1)
    q_win(2, 1)
    for w in (2, 3):
        for si in range(nslots):
            q_win(si, w, eng=nc.gpsimd)
    return {"qt": qt, "kt": kt, "vs": vs, "mbs": mbs}


NWARM = 10  # small (64-wide fp32, 256c) so they never delay real work


def _emit_compute(tc, outs, slots, tiles, expb, warm_ops, ptp, tailp, psum, psacc):
    nc = tc.nc
    from collections import deque

    T = sum(slots)
    npair = T // 2
    nslots = len(slots)
    base = [sum(slots[:s]) for s in range(nslots)]
    last = [base[s] + slots[s] - 1 for s in range(nslots)]

    def slot_of(i):
        for s in range(nslots - 1, -1, -1):
            if i >= base[s]:
                return s
        return 0

    qt, kt, vs, mbs = tiles["qt"], tiles["kt"], tiles["vs"], tiles["mbs"]
    outot, outden = outs["outot"], outs["outden"]

    import heapq

    due = []  # (due_global_pair, seq, tail_fn)
    seq = 0
    pv_q = deque()
    pair_last = [last[s] // 2 for s in range(nslots)]

    def emit_pv(ots, sbc, pair, pt):
        for j in range(2):
            i = 2 * pair + j
            s = slot_of(i)
            psl = slice(j * QW, (j + 1) * QW)
            nc.tensor.matmul(
                ots[s],
                lhsT=vs[:, i * P : (i + 1) * P],
                rhs=pt[:, psl],
                start=(i == base[s]),
                stop=(i == last[s]),
            )
            nc.tensor.matmul(
                sbc,
                lhsT=mbs[:, i * P : (i + 1) * P],
                rhs=pt[:, psl],
                start=(i == 0),
                stop=(i == T - 1),
            )

    def tail_slot(s, qsl, ots, sbc, eng):
        # PSUM cannot DMA directly to DRAM: bounce through SBUF on the
        # (otherwise idle) DVE, then store with a single trigger (its 128
        # descriptors spread over all 16 DMA engines). Triggers go to the
        # idle GpSimd queue mid-kernel and to Sync (free by then) at the end.
        def fn():
            on = tailp.tile([P, QW], FP32, tag=f"on{s}", name=f"on{s}")
            nc.vector.tensor_scalar_add(on, ots[s], 0.0)
            eng.dma_start(outot[s][:, qsl], on)
            if s == nslots - 1:
                dsb = tailp.tile([nslots, QW], FP32, tag="dsb", name="dsb")
                nc.vector.tensor_scalar_add(dsb, sbc[0:nslots, :], 0.0)
                eng.dma_start(outden[:, qsl], dsb)

        return fn

    # PE p-state warm-up: ~16 dependency-free matmuls on zero constants keep
    # the tensor engine continuously busy (targets the st pool; no readers)
    # so it reaches max clock before the first real score matmul.
    wl, wr = warm_ops
    for w in range(NWARM):
        stw = psum.tile([P, 2 * QW], FP32, tag="st", name=f"warm{w}")
        nc.tensor.matmul(stw[:, 0:64], lhsT=wl, rhs=wr, start=True, stop=True)

    gp = 0  # global pair counter across passes
    for ip in range(NPASS):
        qsl = slice(ip * QW, (ip + 1) * QW)
        ots = [
            psacc.tile([P, QW], FP32, tag=f"ot{s}", name=f"ot{s}")
            for s in range(nslots)
        ]
        sbc = psacc.tile([P, QW], FP32, tag="sbc")
        for pair in range(npair):
            while due and due[0][0] <= gp:
                heapq.heappop(due)[2]()
            st = psum.tile([P, 2 * QW], FP32, tag="st")
            for j in range(2):
                i = 2 * pair + j
                s = slot_of(i)
                nc.tensor.matmul(
                    st[:, j * QW : (j + 1) * QW],
                    lhsT=kt[:, i * P : (i + 1) * P],
                    rhs=qt[:, s * SQ + ip * QW : s * SQ + (ip + 1) * QW],
                    start=True,
                    stop=True,
                )
            pt = ptp.tile([P, 2 * QW], F32R, tag="pt")
            nc.scalar.activation(
                pt, st, mybir.ActivationFunctionType.Exp,
                bias=expb, scale=SCALE,
            )
            pv_q.append((ots, sbc, pair, pt))
            if len(pv_q) > 3:
                emit_pv(*pv_q.popleft())
            gp += 1

        # per-slot tails, due one pair after the slot's last PV matmul has
        # drained from pv_q (emitting earlier would read a partial PSUM sum
        # under Tile's program-order semantics)
        for s in range(nslots):
            eng = nc.sync if ip == NPASS - 1 else nc.gpsimd
            heapq.heappush(
                due,
                (ip * npair + pair_last[s] + 4, seq, tail_slot(s, qsl, ots, sbc, eng)),
            )
            seq += 1

    while pv_q:
        emit_pv(*pv_q.popleft())
    while due:
        heapq.heappop(due)[2]()


def _build_kernel(ctx, tc, outs, ins, slots):
    nc = tc.nc
    consts = ctx.enter_context(tc.tile_pool(name="consts", bufs=1))
    big = ctx.enter_context(tc.tile_pool(name="big", bufs=1))
    ptp = ctx.enter_context(tc.tile_pool(name="ptp", bufs=6))
    tailp = ctx.enter_context(tc.tile_pool(name="tailp", bufs=2))
    psum = ctx.enter_context(tc.tile_pool(name="psum", bufs=2, space="PSUM"))
    psacc = ctx.enter_context(tc.tile_pool(name="psacc", bufs=1, space="PSUM"))

    # warm the ACT exp spline table during the initial DMA wait
    warm = consts.tile([P, 1], FP32)
    nc.vector.memset(warm, 0.0)
    nc.scalar.activation(warm, warm, mybir.ActivationFunctionType.Exp)
    expb = consts.tile([P, 1], FP32, name="expb")
    nc.vector.memset(expb, EXP_BIAS)
    # PE warm-up operands: dummy matmuls ramp the tensor engine's DVFS
    # p-state to max while the first input DMAs are still in flight.
    wl = consts.tile([P, P], FP32, name="wl")
    wr = consts.tile([P, 64], FP32, name="wr")
    nc.vector.memset(wl, 0.0)
    nc.vector.memset(wr, 0.0)

    tiles = _emit_loads(tc, ins, slots, big)
    _emit_compute(tc, outs, slots, tiles, expb, (wl, wr), ptp, tailp, psum, psacc)


_NC_CACHE = {}


def _get_nc(slots):
    if slots in _NC_CACHE:
        return _NC_CACHE[slots]
    from contextlib import ExitStack

    T = sum(slots)
    nslots = len(slots)
    nc = bacc.Bacc(
        "TRN2",
        target_bir_lowering=False,
        debug=False,
        enable_asserts=False,
        num_devices=NCORES,
    )
    ins = {
        "qt": nc.dram_tensor("qt", [D, nslots * SQ], F32R, kind="ExternalInput").ap(),
        "kt": nc.dram_tensor("kt", [D, T * P], F32R, kind="ExternalInput").ap(),
        "vm": nc.dram_tensor("vm", [D, T * P], F32R, kind="ExternalInput").ap(),
        "mb": nc.dram_tensor("mb", [D, T * P], F32R, kind="ExternalInput").ap(),
    }
    outs = {
        "outot": nc.dram_tensor(
            "outot", [nslots, D, SQ], FP32, kind="ExternalOutput"
        ).ap(),
        "outden": nc.dram_tensor(
            "outden", [nslots, SQ], FP32, kind="ExternalOutput"
        ).ap(),
    }
    with tile.TileContext(nc) as tc:
        with ExitStack() as ctx:
            _build_kernel(ctx, tc, outs, ins, slots)
    nc.compile()
    _NC_CACHE[slots] = nc
    return nc


LAST_RESULTS = None  # BassKernelResults of the last run (for test harness)


def kernel(q, k, v, valid_len):
    q = np.ascontiguousarray(np.asarray(q, dtype=np.float32))
    k = np.ascontiguousarray(np.asarray(k, dtype=np.float32))
    v = np.ascontiguousarray(np.asarray(v, dtype=np.float32))
    vl = np.asarray(valid_len).astype(np.int64)

    tiles = [int(min((t + P - 1) // P, SK // P)) for t in vl]
    slots, assign = _pick_config(tiles)
    nslots = len(slots)
    T = sum(slots)

    m = (np.arange(SK)[None, :] < vl[:, None]).astype(np.float32)  # [B, SK]
    vm_full = v * m[:, :, None]  # [B, SK, D]
    qT = np.swapaxes(q, 1, 2)  # [B, D, SQ]
    kT = np.swapaxes(k, 1, 2)  # [B, D, SK]

    # ---- unit allocation: slot s of core c, cores assigned in order ----
    slot_cursor = [0] * nslots
    qtA = np.zeros((NCORES, D, nslots * SQ), np.float32)
    ktA = np.zeros((NCORES, D, T * P), np.float32)
    vmA = np.zeros((NCORES, P, T * P), np.float32)
    mbA = np.zeros((NCORES, P, T * P), np.float32)
    base = [sum(slots[:s]) for s in range(nslots)]
    units = []  # (batch, core, slot)
    for b, parts in assign.items():
        k0 = 0
        for s, cnt in parts:
            c = slot_cursor[s]
            slot_cursor[s] += 1
            assert c < NCORES, "unit packing overflow"
            units.append((b, c, s))
            qtA[c][:, s * SQ : (s + 1) * SQ] = qT[b]
            for l in range(cnt):
                g = k0 + l
                vt = base[s] + l
                ktA[c][:, vt * P : (vt + 1) * P] = kT[b][:, g * P : (g + 1) * P]
                vmA[c][:, vt * P : (vt + 1) * P] = vm_full[b][g * P : (g + 1) * P, :]
                mbA[c][:, vt * P + s] = m[b][g * P : (g + 1) * P]
            k0 += cnt

    nc = _get_nc(slots)
    in_maps = [
        {"qt": qtA[c], "kt": ktA[c], "vm": vmA[c], "mb": mbA[c]}
        for c in range(NCORES)
    ]
    tr = int(os.environ.get("KERNEL_TRACE", "0"))
    res = run_bass_kernel_spmd(
        nc,
        in_maps,
        core_ids=list(range(NCORES)),
        trace=tr > 0,
        trace_cores=(list(range(NCORES)) if tr == 2 else [0]) if tr else None,
    )
    global LAST_RESULTS
    LAST_RESULTS = res

    # ---- host merge: out_b = (sum OT_u) / (sum den_u) ----
    out = np.empty((B, SQ, D), np.float32)
    for b in range(B):
        us = [(c, s) for (bb, c, s) in units if bb == b]
        if not us:
            out[b] = v[b].mean(axis=0, keepdims=True)
            continue
        OT = np.zeros((D, SQ), np.float64)
        den = np.zeros((SQ,), np.float64)
        for c, s in us:
            OT += res.results[c]["outot"][s]
            den += res.results[c]["outden"][s]
        out[b] = (OT / den[None, :]).T
    return out.astype(np.float32)


# revision 25
# speedup vs baseline: 1.0312x; 1.0109x over previous
"""Masked dot-product attention on 8 Trainium2 NeuronCores — v2.

Problem: q,k,v [16, 2048, 128] fp32, valid_len [16] int -> out [16, 2048, 128].
out[b] = softmax(mask(q[b] @ k[b].T / sqrt(128), valid_len[b])) @ v[b]

v2 exploits that valid_len masks out entire 128-key tiles: only
ceil(valid_len/128) key tiles per batch contribute (127 of 256 total for the
graded input). Work is rebalanced across cores at KEY-TILE granularity using
flash-style partial attention:

  - Every core runs the SAME program (SPMD requirement): 3 "slot units" of
    fixed tile counts SLOTS=(8,6,4) = 18 virtual key tiles x 4 query passes.
  - A unit = (batch, contiguous range of live key tiles). The host assigns
    units to (core, slot) via bin packing and ships packed K/V/mask tiles plus
    that batch's Q^T per slot. Unused capacity is zero-padded (K=0 -> exp=1,
    V=0/mask=0 -> contributes nothing).
  - The device never normalizes: each unit outputs raw OT (V^T @ P^T partial)
    and its denominator row. The host merges: out_b = (sum OT_u) / (sum den_u)
    (exact — no max subtraction anywhere, so no rescaling is needed).
  - Denominator: each tile's mask matrix mb has its 0/1 mask in COLUMN s
    (slot id), so all 3 slots' denominator matmuls accumulate into a single
    shared PSUM bank (row s = den_s). PSUM = st(2 bufs x 2) + ot(3) + sbc(1)
    = exactly 8 banks.

Per-core per-pass pipeline (transposed layout, f32r matmuls, as v1):
    S^T_i = K_i^T.T @ Q_s^T      (PE, PSUM [k=128, q=512] x2 per pair)
    P^T_i = exp(S^T_i / sqrt(d)) (ScalarE, one [128,1024] inst per pair)
    OT_s += V_i.T  @ P^T_i       (PE accum, start/stop at slot bounds)
    den  += Mb_i.T @ P^T_i       (PE accum into shared bank, col-s banding)
    -> direct DMA store of OT_s (PSUM->DRAM) + den row per (slot, pass)
PV/sbc matmuls trail the score matmuls by 3 pairs (in-order PE queue keeps
busy while ACT computes exp); DMA tails deferred to pair 3 of the next pass
(Tile program-order: a read emitted before the trailing accumulating matmuls
would legally see a partial sum).
"""

import os

import numpy as np

import concourse.tile as tile
from concourse import bacc, mybir
from concourse.bass_utils import run_bass_kernel_spmd

B, SQ, SK, D = 16, 2048, 2048, 128
NCORES = 8
P = 128  # partitions
QW = 512  # query window (one PSUM bank)
NPASS = SQ // QW
SCALE = 1.0 / float(np.sqrt(D))

FP32 = mybir.dt.float32
F32R = mybir.dt.float32r
FP8 = mybir.dt.float8e4  # OCP e4m3, max 240
NPFP8 = mybir.dt.np(FP8)
# exp(s/sqrt(d) + EXP_BIAS): numerator and denominator scale identically, so
# the ratio is exact. -3 keeps e4m3 (max 240) finite up to 8.5-sigma scores
# (observed max 7.44) while keeping dominant weights in the normal range.
EXP_BIAS = -3.0

DEFAULT_SLOTS = (8, 6, 4)


# ---------------- unit packing (host) ----------------

def _pick_config(tiles):
    """Smallest-total 3-slot config that packs `tiles`. (16,16,16) always
    works (every batch fits one unit, 16 batches <= 24 units)."""
    cands = []
    for t1 in range(2, 17):
        for t2 in range(1, t1 + 1):
            for t3 in range(1, t2 + 1):
                if (t1 + t2 + t3) % 2 == 0:
                    cands.append((t1, t2, t3))
    cands.sort(key=lambda c: (sum(c), c[0]))
    for c in cands:
        a = _solve_units_slots(tiles, c)
        if a is not None:
            return c, a
    c = (16, 16, 16)
    return c, _solve_units_slots(tiles, c)


def _solve_units_slots(tiles, slots):
    """Like _solve_units but units are (slot_index) with possibly repeated
    sizes. Returns {batch: [(slot_idx, ntiles)]} using 8 units per slot."""
    nslots = len(slots)
    avail = [NCORES] * nslots
    assign = {b: [] for b in range(len(tiles))}
    order = sorted(range(nslots), key=lambda s: slots[s])  # ascending size
    multi = []
    for b in sorted(range(len(tiles)), key=lambda b: tiles[b]):
        t = tiles[b]
        if t == 0:
            continue
        fit = [s for s in order if slots[s] >= t and avail[s] > 0]
        if fit:
            s = fit[0]
            avail[s] -= 1
            assign[b].append((s, t))
        else:
            multi.append(b)
    for b in sorted(multi, key=lambda b: -tiles[b]):
        rem = tiles[b]
        while rem > 0:
            opts = [s for s in sorted(range(nslots), key=lambda s: -slots[s])
                    if avail[s] > 0]
            if not opts:
                return None
            cover = [s for s in order if slots[s] >= rem and avail[s] > 0]
            s = cover[0] if cover else opts[0]
            avail[s] -= 1
            take = min(slots[s], rem)
            assign[b].append((s, take))
            rem -= take
    return assign


# ---------------- device program ----------------

def _emit_loads(tc, ins, slots, big):
    """Queue input DMAs, ordered by first compute use. All inputs are 2-D
    [P, X] contiguous, so each dma_start is one cheap HWDGE trigger (~0.6us
    on its sequencer) whose 128 descriptors spread over all 16 DMA engines.
    Triggers are split between the Sync queue (kt/qt: pass-0 critical) and
    the idle GpSimd queue (vs/mbs and late q windows)."""
    nc = tc.nc
    T = sum(slots)
    qT, kT, vm, mb = ins["qt"], ins["kt"], ins["vm"], ins["mb"]
    nslots = len(slots)
    qt = big.tile([P, nslots * SQ], F32R, tag="qt")
    kt = big.tile([P, T * P], F32R, tag="kt")
    vs = big.tile([P, T * P], F32R, tag="vs")
    mbs = big.tile([P, T * P], F32R, tag="mbs")

    def q_win(s, w, eng=None):
        fs = slice(s * SQ + w * QW, s * SQ + (w + 1) * QW)
        (eng or nc.sync).dma_start(qt[:, fs], qT[:, fs])

    def k_chunk(c0, c1):
        fs = slice(c0 * P, c1 * P)
        nc.sync.dma_start(kt[:, fs], kT[:, fs])

    def vm_chunk(c0, c1, eng=None):
        fs = slice(c0 * P, c1 * P)
        (eng or nc.gpsimd).dma_start(vs[:, fs], vm[:, fs])
        (eng or nc.gpsimd).dma_start(mbs[:, fs], mb[:, fs])

    k_chunk(0, 4)
    q_win(0, 0)
    vm_chunk(0, 4, eng=nc.sync)  # pv of pair 0 trails scores by only ~3 pairs
    vm_chunk(4, T // 2)
    k_chunk(4, 12)
    q_win(1, 0)
    q_win(2, 0)
    vm_chunk(T // 2, T)
    k_chunk(12, T)
    q_win(0, 1)
    q_win(1, 1)
    q_win(2, 1)
    for w in (2, 3):
        for si in range(nslots):
            q_win(si, w, eng=nc.gpsimd)
    return {"qt": qt, "kt": kt, "vs": vs, "mbs": mbs}


NWARM = 10  # small (64-wide fp32, 256c) so they never delay real work


def _emit_compute(tc, outs, slots, tiles, expb, warm_ops, ptp, tailp, psum, psacc):
    nc = tc.nc
    from collections import deque

    T = sum(slots)
    npair = T // 2
    nslots = len(slots)
    base = [sum(slots[:s]) for s in range(nslots)]
    last = [base[s] + slots[s] - 1 for s in range(nslots)]

    def slot_of(i):
        for s in range(nslots - 1, -1, -1):
            if i >= base[s]:
                return s
        return 0

    qt, kt, vs, mbs = tiles["qt"], tiles["kt"], tiles["vs"], tiles["mbs"]
    outot, outden = outs["outot"], outs["outden"]

    import heapq

    due = []  # (due_global_pair, seq, tail_fn)
    seq = 0
    pv_q = deque()
    pair_last = [last[s] // 2 for s in range(nslots)]

    def emit_pv(ots, sbc, pair, pt):
        for j in range(2):
            i = 2 * pair + j
            s = slot_of(i)
            psl = slice(j * QW, (j + 1) * QW)
            nc.tensor.matmul(
                ots[s],
                lhsT=vs[:, i * P : (i + 1) * P],
                rhs=pt[:, psl],
                start=(i == base[s]),
                stop=(i == last[s]),
            )
            nc.tensor.matmul(
                sbc,
                lhsT=mbs[:, i * P : (i + 1) * P],
                rhs=pt[:, psl],
                start=(i == 0),
                stop=(i == T - 1),
            )

    def tail_slot(s, qsl, ots, sbc, eng):
        # PSUM cannot DMA directly to DRAM: bounce through SBUF on the
        # (otherwise idle) DVE, then store with a single trigger (its 128
        # descriptors spread over all 16 DMA engines). Triggers go to the
        # idle GpSimd queue mid-kernel and to Sync (free by then) at the end.
        def fn():
            on = tailp.tile([P, QW], FP32, tag=f"on{s}", name=f"on{s}")
            nc.vector.tensor_scalar_add(on, ots[s], 0.0)
            eng.dma_start(outot[s][:, qsl], on)
            if s == nslots - 1:
                dsb = tailp.tile([nslots, QW], FP32, tag="dsb", name="dsb")
                nc.vector.tensor_scalar_add(dsb, sbc[0:nslots, :], 0.0)
                eng.dma_start(outden[:, qsl], dsb)

        return fn

    # PE p-state warm-up: ~16 dependency-free matmuls on zero constants keep
    # the tensor engine continuously busy (targets the st pool; no readers)
    # so it reaches max clock before the first real score matmul.
    wl, wr = warm_ops
    for w in range(NWARM):
        stw = psum.tile([P, 2 * QW], FP32, tag="st", name=f"warm{w}")
        nc.tensor.matmul(stw[:, 0:64], lhsT=wl, rhs=wr, start=True, stop=True)

    gp = 0  # global pair counter across passes
    for ip in range(NPASS):
        qsl = slice(ip * QW, (ip + 1) * QW)
        ots = [
            psacc.tile([P, QW], FP32, tag=f"ot{s}", name=f"ot{s}")
            for s in range(nslots)
        ]
        sbc = psacc.tile([P, QW], FP32, tag="sbc")
        for pair in range(npair):
            while due and due[0][0] <= gp:
                heapq.heappop(due)[2]()
            st = psum.tile([P, 2 * QW], FP32, tag="st")
            for j in range(2):
                i = 2 * pair + j
                s = slot_of(i)
                nc.tensor.matmul(
                    st[:, j * QW : (j + 1) * QW],
                    lhsT=kt[:, i * P : (i + 1) * P],
                    rhs=qt[:, s * SQ + ip * QW : s * SQ + (ip + 1) * QW],
                    start=True,
                    stop=True,
                )
            pt = ptp.tile([P, 2 * QW], F32R, tag="pt")
            nc.scalar.activation(
                pt, st, mybir.ActivationFunctionType.Exp,
                bias=expb, scale=SCALE,
            )
            pv_q.append((ots, sbc, pair, pt))
            if len(pv_q) > 3:
                emit_pv(*pv_q.popleft())
            gp += 1

        # per-slot tails, due one pair after the slot's last PV matmul has
        # drained from pv_q (emitting earlier would read a partial PSUM sum
        # under Tile's program-order semantics)
        for s in range(nslots):
            eng = nc.sync if ip == NPASS - 1 else nc.gpsimd
            heapq.heappush(
                due,
                (ip * npair + pair_last[s] + 4, seq, tail_slot(s, qsl, ots, sbc, eng)),
            )
            seq += 1

    while pv_q:
        emit_pv(*pv_q.popleft())
    while due:
        heapq.heappop(due)[2]()


def _build_kernel(ctx, tc, outs, ins, slots):
    nc = tc.nc
    consts = ctx.enter_context(tc.tile_pool(name="consts", bufs=1))
    big = ctx.enter_context(tc.tile_pool(name="big", bufs=1))
    ptp = ctx.enter_context(tc.tile_pool(name="ptp", bufs=6))
    tailp = ctx.enter_context(tc.tile_pool(name="tailp", bufs=2))
    psum = ctx.enter_context(tc.tile_pool(name="psum", bufs=2, space="PSUM"))
    psacc = ctx.enter_context(tc.tile_pool(name="psacc", bufs=1, space="PSUM"))

    # warm the ACT exp spline table during the initial DMA wait
    warm = consts.tile([P, 1], FP32)
    nc.vector.memset(warm, 0.0)
    nc.scalar.activation(warm, warm, mybir.ActivationFunctionType.Exp)
    expb = consts.tile([P, 1], FP32, name="expb")
    nc.vector.memset(expb, EXP_BIAS)
    # PE warm-up operands: dummy matmuls ramp the tensor engine's DVFS
    # p-state to max while the first input DMAs are still in flight.
    wl = consts.tile([P, P], FP32, name="wl")
    wr = consts.tile([P, 64], FP32, name="wr")
    nc.vector.memset(wl, 0.0)
    nc.vector.memset(wr, 0.0)

    tiles = _emit_loads(tc, ins, slots, big)
    _emit_compute(tc, outs, slots, tiles, expb, (wl, wr), ptp, tailp, psum, psacc)


_NC_CACHE = {}


def _get_nc(slots):
    if slots in _NC_CACHE:
        return _NC_CACHE[slots]
    from contextlib import ExitStack

    T = sum(slots)
    nslots = len(slots)
    nc = bacc.Bacc(
        "TRN2",
        target_bir_lowering=False,
        debug=False,
        enable_asserts=False,
        num_devices=NCORES,
    )
    ins = {
        "qt": nc.dram_tensor("qt", [D, nslots * SQ], F32R, kind="ExternalInput").ap(),
        "kt": nc.dram_tensor("kt", [D, T * P], F32R, kind="ExternalInput").ap(),
        "vm": nc.dram_tensor("vm", [D, T * P], F32R, kind="ExternalInput").ap(),
        "mb": nc.dram_tensor("mb", [D, T * P], F32R, kind="ExternalInput").ap(),
    }
    outs = {
        "outot": nc.dram_tensor(
            "outot", [nslots, D, SQ], FP32, kind="ExternalOutput"
        ).ap(),
        "outden": nc.dram_tensor(
            "outden", [nslots, SQ], FP32, kind="ExternalOutput"
        ).ap(),
    }
    with tile.TileContext(nc) as tc:
        with ExitStack() as ctx:
            _build_kernel(ctx, tc, outs, ins, slots)
    nc.compile()
    _NC_CACHE[slots] = nc
    return nc


LAST_RESULTS = None  # BassKernelResults of the last run (for test harness)


def kernel(q, k, v, valid_len):
    q = np.ascontiguousarray(np.asarray(q, dtype=np.float32))
    k = np.ascontiguousarray(np.asarray(k, dtype=np.float32))
    v = np.ascontiguousarray(np.asarray(v, dtype=np.float32))
    vl = np.asarray(valid_len).astype(np.int64)

    tiles = [int(min((t + P - 1) // P, SK // P)) for t in vl]
    slots, assign = _pick_config(tiles)
    nslots = len(slots)
    T = sum(slots)

    m = (np.arange(SK)[None, :] < vl[:, None]).astype(np.float32)  # [B, SK]
    vm_full = v * m[:, :, None]  # [B, SK, D]
    qT = np.swapaxes(q, 1, 2)  # [B, D, SQ]
    kT = np.swapaxes(k, 1, 2)  # [B, D, SK]

    # ---- unit allocation: slot s of core c, cores assigned in order ----
    slot_cursor = [0] * nslots
    qtA = np.zeros((NCORES, D, nslots * SQ), np.float32)
    ktA = np.zeros((NCORES, D, T * P), np.float32)
    vmA = np.zeros((NCORES, P, T * P), np.float32)
    mbA = np.zeros((NCORES, P, T * P), np.float32)
    base = [sum(slots[:s]) for s in range(nslots)]
    units = []  # (batch, core, slot)
    for b, parts in assign.items():
        k0 = 0
        for s, cnt in parts:
            c = slot_cursor[s]
            slot_cursor[s] += 1
            assert c < NCORES, "unit packing overflow"
            units.append((b, c, s))
            qtA[c][:, s * SQ : (s + 1) * SQ] = qT[b]
            for l in range(cnt):
                g = k0 + l
                vt = base[s] + l
                ktA[c][:, vt * P : (vt + 1) * P] = kT[b][:, g * P : (g + 1) * P]
                vmA[c][:, vt * P : (vt + 1) * P] = vm_full[b][g * P : (g + 1) * P, :]
                mbA[c][:, vt * P + s] = m[b][g * P : (g + 1) * P]
            k0 += cnt

    nc = _get_nc(slots)
    in_maps = [
        {"qt": qtA[c], "kt": ktA[c], "vm": vmA[c], "mb": mbA[c]}
        for c in range(NCORES)
    ]
    tr = int(os.environ.get("KERNEL_TRACE", "0"))
    res = run_bass_kernel_spmd(
        nc,
        in_maps,
        core_ids=list(range(NCORES)),
        trace=tr > 0,
        trace_cores=(list(range(NCORES)) if tr == 2 else [0]) if tr else None,
    )
    global LAST_RESULTS
    LAST_RESULTS = res

    # ---- host merge: out_b = (sum OT_u) / (sum den_u) ----
    out = np.empty((B, SQ, D), np.float32)
    for b in range(B):
        us = [(c, s) for (bb, c, s) in units if bb == b]
        if not us:
            out[b] = v[b].mean(axis=0, keepdims=True)
            continue
        OT = np.zeros((D, SQ), np.float64)
        den = np.zeros((SQ,), np.float64)
        for c, s in us:
            OT += res.results[c]["outot"][s]
            den += res.results[c]["outden"][s]
        out[b] = (OT / den[None, :]).T
    return out.astype(np.float32)
